# revision 10
# baseline (speedup 1.0000x reference)
"""Trainium2 Bass kernel for nn_EntropyLoss (retrieval_knn).

Computes var([E(f1)-E(f0), E(f2)-E(f1)], ddof=1) where
E(f) = log(1 + sum_b sum_i r_ball[b, i]) and r_ball[b, i] is the K-th
nearest-neighbor distance (K = C//10 = 51, i.e. 52nd smallest including
the self-distance 0) among the C=512 channel vectors (dim H*W = 4096)
of sample b.

v2 design (vs the 249us max8/match_replace baseline):
  PE   : per-unit Gram matrix with SYMMETRY: only diagonal+upper block
         columns are computed directly (fp16, K=128 chunks); lower
         blocks are filled by transpose-mode matmuls of the (biased)
         fp16 SBUF copy of earlier row-blocks plus a K=1 per-partition
         compensation matmul (-b_I[p]) and the global per-column bias
         row (+b_j).  ~35% fewer PE cycles.
  sel  : per-row rank-52 threshold found by an 8-step dyadic bisection
         on the count #(m > t).  Counts run on BOTH engines:
           - DVE blocks: tensor_scalar (m + NT > -P) with accum_out
           - ACT blocks: activation Sign(m + NT) with accum_out
         Threshold state NT [128,24] is updated in lockstep (1-2 small
         DVE ops per iter for all 24 blocks).  A final guarded pass at
         t_fin = t_8 + g (t* <= t_fin <= t* + 2*Delta_8 + g) yields an
         exact count c <= 51 and the mask (m > t_fin); two max8 rounds
         on the masked row then give ranks c+1..c+16 which contain rank
         52 (c >= 36 whp).  The host picks column 51 - c.
  host : d2 = sq_i + 4096 - 2*m52, r = sqrt(max(d2,0)), log/var tail.
"""
import sys

for _p in ("/opt/trn_rl_repo", "/root/.axon_site/_ro/trn_rl_repo"):
    if _p not in sys.path:
        sys.path.insert(0, _p)

import numpy as np

from concourse import bacc, mybir
from concourse.alu_op_type import AluOpType
from concourse.tile import TileContext
from concourse.bass_utils import run_bass_kernel_spmd

B, C, H, W = 16, 512, 64, 64
D = H * W  # 4096
K = C // 10  # 51 -> want 52nd smallest distance per row
RANK = K + 1  # 52
N_CORES = 8
N_TENSORS = 3
UNITS = N_TENSORS * B  # 48
UPC = UNITS // N_CORES  # units per core = 6
KCHUNKS = D // 128  # 32
RBLK = C // 128  # 4 row blocks per unit
NBLK = UPC * RBLK  # 24 blocks per core

# --- bisection parameters ---
N_ITERS = 6
DELTAS = [32.0, 16.0, 8.0, 4.0, 2.0, 1.0]  # W0 = 64 bracket
GUARD = 1.0  # must be >= DELTAS[-1]
P_FULL = sum(DELTAS)  # 63.75
Z_QUANT = 1.2885  # Phi^-1(1 - 50.5/511)
NEGBIG = -60000.0
N_EXTRACT = 16  # two max8 rounds -> ranks c+1..c+16

N_ACT = UPC * 3  # blocks bisected via ACT counts (i in {1,2,3}); i==0 classic DVE
ROUNDS = RANK // 8 + (1 if RANK % 8 else 0)  # 7 classic rounds
SEL_COL = (RANK - 1) % 8  # rank-52 column within classic round 7
MASKSHIFT = -50000.0
SYM = True  # symmetric PE (transpose lower blocks)
FP8 = False  # fp8e4m3 direct chunks (DoubleRow for I<2, plain fp8 for I>=2)

TRACE = False  # test.py flips this for profiling
_LAST = {}  # debug stash

DMA_SPLIT = 4  # xt DMAs per sample




def _build_program(repeat=1, ablate=(), loop_n=None):
    """ablate: subset of {"sel", "ext", "mm", "dma"} for timing ablations."""
    nc = bacc.Bacc("TRN2", target_bir_lowering=False, debug=False)
    f16, f32 = mybir.dt.float16, mybir.dt.float32
    xdt = mybir.dt.float8e4 if FP8 else f16

    xt_d = nc.dram_tensor("xt", [UPC, 128, KCHUNKS * C], xdt, kind="ExternalInput")
    # sqn[s, j] = fp16(2048 - sq[s, j]/2) (per-column bias row)
    sqn_d = nc.dram_tensor("sqn", [UPC, C], f16, kind="ExternalInput")
    # nsq = -sqn (per-partition compensation for transposed blocks)
    nsq_d = nc.dram_tensor("nsq", [UPC, C], f16, kind="ExternalInput")
    # nt0[p, col] = -t0 for row (u, 128*I + p), col = _col_of(u, I)
    nt0_d = nc.dram_tensor("nt0", [128, NBLK], f32, kind="ExternalInput")
    ident_d = nc.dram_tensor("ident", [128, 128], f32, kind="ExternalInput")

    out_d = nc.dram_tensor("msel", [128, N_ACT * N_EXTRACT], f16, kind="ExternalOutput")
    outc_d = nc.dram_tensor("mselc", [128, UPC * 8], f32, kind="ExternalOutput")
    cnt_d = nc.dram_tensor("cnt", [128, NBLK], f32, kind="ExternalOutput")

    kper = KCHUNKS // DMA_SPLIT  # k-chunks per DMA piece
    xt_view = xt_d.ap().rearrange("s p (d k c) -> s p d k c", d=DMA_SPLIT, k=kper)

    with TileContext(nc) as tc:
        with (
            tc.tile_pool(name="xpool", bufs=2 * DMA_SPLIT) as xpool,
            tc.tile_pool(name="consts", bufs=1) as consts,
            tc.tile_pool(name="msbp", bufs=20) as msbp,
            tc.tile_pool(name="state", bufs=2) as statep,
            tc.tile_pool(name="apool", bufs=3) as apool,
            tc.tile_pool(name="scrd", bufs=3) as scrd,
            tc.tile_pool(name="scra", bufs=3) as scra,
            tc.tile_pool(name="mskp", bufs=3) as mskp,
            tc.tile_pool(name="smalls", bufs=6) as smallp,
            tc.tile_pool(name="gps", bufs=6, space="PSUM") as gps,
        ):
            ones1 = consts.tile([1, 128], f16)
            nc.vector.memset(ones1, 1.0)
            ident = consts.tile([128, 128], f32)
            nc.sync.dma_start(out=ident, in_=ident_d.ap())
            OUT = consts.tile([128, N_ACT * N_EXTRACT], f16)
            OUTC = consts.tile([128, UPC * 8], f32)
            CNT = consts.tile([128, NBLK], f32)
            nc.vector.memset(CNT, 0.0)
            sqn_all = consts.tile([1, UPC * C], f16)
            nc.sync.dma_start(
                out=sqn_all, in_=sqn_d.ap().rearrange("s c -> (s c)").unsqueeze(0)
            )
            nsq_all = consts.tile([1, UPC * C], f16)
            nc.sync.dma_start(
                out=nsq_all, in_=nsq_d.ap().rearrange("s c -> (s c)").unsqueeze(0)
            )

            def pipeline_body(_iv=None):
                NT = statep.tile([128, NBLK], f32, tag="nt")
                nc.sync.dma_start(out=NT, in_=nt0_d.ap())
                NTv = NT.rearrange("p (u i) -> p u i", u=UPC)
                sign_f = mybir.ActivationFunctionType.Sign

                msb = [[None] * RBLK for _ in range(UPC)]
                xparts_cached = [None]

                def emit_unit_pe(s):
                    if "dma" in ablate and xparts_cached[0] is not None:
                        xparts = xparts_cached[0]
                    else:
                        xparts = []
                        for d in range(DMA_SPLIT):
                            xp = xpool.tile([128, kper, C], xdt, tag="xts")
                            nc.sync.dma_start(out=xp, in_=xt_view[s, :, d])
                            xparts.append(xp)
                        xparts_cached[0] = xparts

                    sqn = sqn_all[:, s * C : (s + 1) * C]
                    nsq = nsq_all[:, s * C : (s + 1) * C]

                    for I in range(RBLK):
                        g_ps = gps.tile([128, C], f32, tag="g")
                        nc.tensor.matmul(
                            out=g_ps, lhsT=ones1, rhs=sqn, start=True, stop=False
                        )
                        c0 = 128 * I if SYM else 0
                        nkc = 1 if "mm" in ablate else KCHUNKS
                        ntr = I if (SYM and "mm" not in ablate) else 0
                        use_dr = FP8 and I < 2 and nkc == KCHUNKS
                        if use_dr:
                            for kk in range(KCHUNKS // 2):
                                k2 = 2 * kk
                                xp = xparts[k2 // kper]
                                lo = k2 % kper
                                nc.tensor.matmul(
                                    out=g_ps[:, c0:C],
                                    lhsT=xp[:, lo : lo + 2, 128 * I : 128 * (I + 1)],
                                    rhs=xp[:, lo : lo + 2, c0:C],
                                    start=False,
                                    stop=(kk == KCHUNKS // 2 - 1 and ntr == 0),
                                    perf_mode=mybir.MatmulPerfMode.DoubleRow,
                                )
                        else:
                            for k in range(nkc):
                                xp = xparts[k // kper]
                                kk = k % kper
                                nc.tensor.matmul(
                                    out=g_ps[:, c0:C],
                                    lhsT=xp[:, kk, 128 * I : 128 * (I + 1)],
                                    rhs=xp[:, kk, c0:C],
                                    start=False,
                                    stop=(k == nkc - 1 and ntr == 0),
                                )
                        for J in range(ntr):
                            nc.tensor.matmul(
                                out=g_ps[:, 128 * J : 128 * (J + 1)],
                                lhsT=msb[s][J][:, 128 * I : 128 * (I + 1)],
                                rhs=ident,
                                is_transpose=True,
                                start=False,
                                stop=False,
                            )
                            nc.tensor.matmul(
                                out=g_ps[:, 128 * J : 128 * (J + 1)],
                                lhsT=nsq[:, 128 * I : 128 * (I + 1)],
                                rhs=ones1,
                                start=False,
                                stop=(J == ntr - 1),
                            )
                        m = msbp.tile([128, C], f32, tag="m")
                        nc.scalar.copy(out=m, in_=g_ps)
                        msb[s][I] = m

                ntfs = [None] * UPC

                def emit_unit_bisect(s):
                    if "sel" in ablate:
                        return
                    for it in range(N_ITERS):
                        A = apool.tile([128, RBLK - 1], f32, tag="acc")
                        for I in range(1, RBLK):
                            scr = scra.tile([128, C], f32, tag="scra")
                            nc.scalar.activation(
                                out=scr,
                                in_=msb[s][I],
                                func=sign_f,
                                bias=NTv[:, s, I : I + 1],
                                accum_out=A[:, I - 1 : I],
                            )
                        bt = smallp.tile([128, RBLK - 1], f32, tag="b")
                        nc.gpsimd.tensor_scalar(
                            out=bt, in0=A, scalar1=-409.0, scalar2=None,
                            op0=AluOpType.is_gt,
                        )
                        dlt = DELTAS[it]
                        ut = smallp.tile([128, RBLK - 1], f32, tag="u")
                        nc.gpsimd.tensor_scalar(
                            out=ut, in0=bt, scalar1=-2.0 * dlt, scalar2=dlt,
                            op0=AluOpType.mult, op1=AluOpType.add,
                        )
                        nc.gpsimd.tensor_add(
                            NTv[:, s, 1:RBLK], NTv[:, s, 1:RBLK], ut
                        )

                    ntf = smallp.tile([128, RBLK - 1], f32, tag="ntf")
                    nc.gpsimd.tensor_scalar(
                        out=ntf, in0=NTv[:, s, 1:RBLK], scalar1=-GUARD,
                        scalar2=None, op0=AluOpType.add,
                    )
                    ntfs[s] = ntf

                def emit_unit_extract(s):
                    if "sel" in ablate or "ext" in ablate:
                        return
                    # classic 7-round max8/match_replace for i == 0
                    if "cls" not in ablate:
                        m = msb[s][0]
                        mw = mskp.tile([128, C], f32, tag="mskc")
                        nc.vector.tensor_copy(mw, m)
                        for r in range(ROUNDS):
                            if r == ROUNDS - 1:
                                nc.vector.max(
                                    out=OUTC[:, s * 8 : s * 8 + 8], in_=mw
                                )
                            else:
                                mx = smallp.tile([128, 8], f32, tag="mx")
                                nc.vector.max(out=mx, in_=mw)
                                nc.vector.match_replace(
                                    out=mw, in_to_replace=mx, in_values=mw,
                                    imm_value=-1e30,
                                )

                    # final guarded pass + 2-round extraction
                    ntf = ntfs[s]
                    for I in range(1, RBLK):
                        col = s * RBLK + I
                        acol = s * 3 + (I - 1)
                        m = msb[s][I]
                        sc = scra.tile([128, C], f32, tag="scra")
                        nc.scalar.activation(
                            out=sc,
                            in_=m,
                            func=sign_f,
                            bias=ntf[:, I - 1 : I],
                            accum_out=CNT[:, col : col + 1],
                        )
                        nc.vector.tensor_scalar_max(sc, sc, 0.0)
                        msk = mskp.tile([128, C], f16, tag="msk")
                        nc.vector.scalar_tensor_tensor(
                            out=msk, in0=sc, scalar=MASKSHIFT, in1=m,
                            op0=AluOpType.mult, op1=AluOpType.add,
                        )
                        o0 = acol * N_EXTRACT
                        nc.vector.max(out=OUT[:, o0 : o0 + 8], in_=msk)
                        nc.vector.match_replace(
                            out=msk,
                            in_to_replace=OUT[:, o0 : o0 + 8],
                            in_values=msk,
                            imm_value=NEGBIG,
                        )
                        nc.vector.max(out=OUT[:, o0 + 8 : o0 + 16], in_=msk)

                # software pipeline: PE(u) || bisect(u-1) || extract(u-2)
                for s in range(UPC):
                    emit_unit_pe(s)
                    if s >= 2:
                        emit_unit_extract(s - 2)
                    if s >= 1:
                        emit_unit_bisect(s - 1)
                emit_unit_bisect(UPC - 1)
                emit_unit_extract(UPC - 2)
                emit_unit_extract(UPC - 1)

                if "sel" not in ablate and "ext" not in ablate:
                    nc.sync.dma_start(out=out_d.ap(), in_=OUT)
                    nc.sync.dma_start(out=outc_d.ap(), in_=OUTC)
                    nc.sync.dma_start(out=cnt_d.ap(), in_=CNT)

            if loop_n is not None:
                with tc.For_i(0, loop_n, 1) as _iv:
                    pipeline_body(_iv)
            else:
                for _rep in range(repeat):
                    pipeline_body()

    nc.compile()
    return nc


_PROGRAM = None


def _host_prep(feats):
    """feats: [UNITS, C, D] float32. Returns xt, sqn16, nsq16, nt0, sq64."""
    sq64 = np.einsum("ucd,ucd->uc", feats, feats, dtype=np.float64, casting="safe")
    sqn16 = (2048.0 - sq64 / 2.0).astype(np.float16)

    from concourse import mybir as _mb

    xdt_np = _mb.dt.np(_mb.dt.float8e4) if FP8 else np.float16
    xt = np.ascontiguousarray(
        feats.astype(xdt_np)
        .transpose(0, 2, 1)
        .reshape(UNITS, KCHUNKS, 128, C)
        .transpose(0, 2, 1, 3)
        .reshape(UNITS, 128, KCHUNKS * C)
    )

    # per-row threshold guess t0 = mu + z * sigma (bracket +-64 is ample)
    s32 = feats.sum(axis=1, dtype=np.float32)  # [U, D]
    dot = np.einsum("ucd,ud->uc", feats, s32)  # sum_j G_ij (incl self)
    Bv = sqn16.astype(np.float64)
    muG = (dot - sq64) / (C - 1)
    mub = (Bv.sum(axis=1, keepdims=True) - Bv) / (C - 1)
    varb = Bv.var(axis=1, keepdims=True)
    sigma = np.sqrt(sq64 + varb)
    t0 = muG + mub + Z_QUANT * sigma  # [U, C]
    nt0 = (-t0).astype(np.float32)
    return xt, sqn16, nt0, sq64


def _nt0_dev_layout(nt0_core):
    """nt0_core: [UPC, C] -> [128, NBLK], natural col = u*RBLK + i."""
    out = np.empty((128, NBLK), dtype=np.float32)
    for u in range(UPC):
        for i in range(RBLK):
            out[:, u * RBLK + i] = nt0_core[u, 128 * i : 128 * (i + 1)]
    return out


def kernel(feat0, feat1, feat2):
    global _PROGRAM
    feats = np.stack(
        [np.asarray(f).reshape(B, C, D) for f in (feat0, feat1, feat2)]
    ).reshape(UNITS, C, D)

    xt, sqn16, nt0, sq64 = _host_prep(feats)
    ident = np.eye(128, dtype=np.float32)

    if _PROGRAM is None:
        _PROGRAM = _build_program()
    nc = _PROGRAM
    in_maps = [
        {
            "xt": xt[c * UPC : (c + 1) * UPC],
            "sqn": sqn16[c * UPC : (c + 1) * UPC],
            "nsq": -sqn16[c * UPC : (c + 1) * UPC],
            "nt0": _nt0_dev_layout(nt0[c * UPC : (c + 1) * UPC]),
            "ident": ident,
        }
        for c in range(N_CORES)
    ]
    out = run_bass_kernel_spmd(
        nc, in_maps, core_ids=list(range(N_CORES)), trace=TRACE
    )
    _LAST.clear()
    _LAST["results"] = out

    m52 = np.empty((UNITS, C), dtype=np.float64)
    nclip = 0
    for core in range(N_CORES):
        sel = out.results[core]["msel"].astype(np.float64)  # [128, N_ACT*16]
        selc = out.results[core]["mselc"].astype(np.float64)  # [128, UPC*8]
        araw = out.results[core]["cnt"].astype(np.float64)  # [128, NBLK]
        for u in range(UPC):
            for i in range(RBLK):
                if i == 0:
                    vals = selc[:, u * 8 + SEL_COL]
                else:
                    a = araw[:, u * RBLK + i]
                    c = np.floor((a + C) / 2 + 0.25)
                    idx = np.rint(RANK - 1 - c).astype(np.int64)
                    bad = (idx < 0) | (idx >= N_EXTRACT)
                    nclip += int(bad.sum())
                    idx = np.clip(idx, 0, N_EXTRACT - 1)
                    acol = u * 3 + (i - 1)
                    vals = sel[np.arange(128), acol * N_EXTRACT + idx]
                m52[core * UPC + u, 128 * i : 128 * (i + 1)] = vals
    _LAST["nclip"] = nclip
    if nclip:
        print(f"WARNING: {nclip} rows clipped rank index", file=sys.stderr)

    d2 = sq64 + 4096.0 - 2.0 * m52
    r = np.sqrt(np.clip(d2, 0.0, None))  # [UNITS, C]
    _LAST["r"] = r
    sums = r.reshape(N_TENSORS, B * C).sum(axis=1)
    e = np.log(sums + 1.0)
    deltas = np.array([e[1] - e[0], e[2] - e[1]])
    var = deltas.var(ddof=1)
    return np.asarray(var, dtype=np.float32)
